# revision 13
# baseline (speedup 1.0000x reference)
"""DenseGATConv-style GNN message passing kernel for Trainium2 (Bass/Tile).

Math (per graph b):
    e      = w_edge[edge_attr[b]]            # [N, N] gather from 4-entry table
    adj_w  = adj[b] * e                      # weighted adjacency
    agg    = adj_w @ x[b]                    # [N, C]
    out[b] = agg @ W_rel + b_rel + x[b] @ W_root

Key tricks:
  * The 4-entry gather w_edge[a], a in {0,1,2,3}, equals the cubic polynomial
    through the 4 points, evaluated in factored form
        p(a)/c3 = (a - r) * ((a + h)^2 + v2)
    with one Square op + two fused scalar_tensor_tensor ops; c3 is folded
    into W_rel on the host.  Squares alternate between ScalarE and GpSimd
    (as (a - p)*a) to balance engine load.
  * All on-chip transposes are REGULAR matmuls against a fp16 identity
    (out = lhsT^T @ I).  Unlike transpose-mode matmuls these pipeline
    back-to-back (~4x faster) and keep the PE HAM clock warm.
  * adj_w is fp16 on-chip: halves SBUF pressure and speeds PE weight loads.
    fp16 keeps rel-err ~1e-3, far under the 2e-2 gate.
  * b_rel enters as a K=1 matmul from constant [1,64]x[1,512] tiles.
  * The aggregation runs in transposed layout (agg^T = sum_j x_j^T @ awT_j),
    per half-graph (512 rows) so tails overlap the next half's compute.

Sharding: data-parallel over batch B=16 across 8 cores (2 graphs/core);
weights replicated.
"""

import sys
from contextlib import ExitStack

sys.path.insert(0, "/opt/trn_rl_repo")

import numpy as np

_B, _N, _C = 16, 1024, 64
_NCORES = 8
_G = _B // _NCORES  # graphs per core
_P = 128
_NT = _N // _P  # 128-row tiles per graph
_H = 512  # rows per half-graph

# Module-level knobs (test.py may flip these before calling kernel()).
TRACE = False
# Which engine computes the quadratic factor per 2-row-tile band index:
# "act" = all ScalarE Square; "gpsimd" = all GpSimd STT; "mix" = alternate.
SQUARE_SPLIT = "mix"
MM_DTYPE = "float16"
LAST_RESULTS = None  # BassKernelResults of the most recent run (for test.py)

_BUILD_CACHE = {}


def _poly_coeffs(w_edge):
    """Cubic through (k, w_edge[k]) for k=0..3, float64. Returns c0..c3."""
    w = np.asarray(w_edge, dtype=np.float64).reshape(4)
    V = np.vander(np.arange(4.0), 4, increasing=True)
    c = np.linalg.solve(V, w)
    return c  # [c0, c1, c2, c3]


def _chain_params(w_edge):
    """Pick the elementwise chain and host-folded scale from w_edge values.

    Returns (mode, params, lead) where `lead` multiplies W_rel on the host and
    the device computes adj_w/lead.
    """
    c0, c1, c2, c3 = _poly_coeffs(w_edge)
    scale = max(np.max(np.abs(np.asarray(w_edge, dtype=np.float64))), 1e-30)
    tol = 1e-7 * scale
    if abs(c3) > tol:
        # monic cubic a^3 + A a^2 + B a + C = (a - r)(a^2 + p a + q)
        A, Bc, Cc = c2 / c3, c1 / c3, c0 / c3
        roots = np.roots([1.0, A, Bc, Cc])
        r = float(np.real(roots[np.argmin(np.abs(np.imag(roots)))]))
        p = A + r
        q = Bc + p * r
        return "cubic", dict(r=r, p=p, q=q, h=p / 2.0, v2=q - p * p / 4.0), c3
    if abs(c2) > tol:
        p2, q2 = c1 / c2, c0 / c2
        return "quad", dict(p=p2, q=q2, h=p2 / 2.0, v2=q2 - p2 * p2 / 4.0), c2
    if abs(c1) > tol:
        return "linear", dict(r=-c0 / c1), c1
    return "const", dict(), c0


def _emit_half(nc, pools, g, half, dram, xs_mm, xT, mode, params, square_split):
    from concourse import mybir

    OP = mybir.AluOpType
    AF = mybir.ActivationFunctionType
    f32 = pools["f32"]
    mmdt = pools["mmdt"]
    adj_d, ea_d, out_d = dram["adj"], dram["ea"], dram["out"]
    ident_m = pools["ident_m"]

    import contextlib

    # x^T columns for this half (root-term operand), via regular-mm transpose
    p_xT = pools["ps_xt"].tile([_C, _H], f32, tag="ps_xt")
    for k in range(4):
        jt = 4 * half + k
        nc.tensor.matmul(
            p_xT[:, k * _P : (k + 1) * _P],
            lhsT=xs_mm[:, jt * _C : (jt + 1) * _C],
            rhs=ident_m[:, :_P],
            start=True, stop=True,
        )
    nc.scalar.copy(out=xT[:, half * _H : (half + 1) * _H], in_=p_xT[:])

    # DMA + elementwise chain: per pair of row-tiles ("band", 1 MiB each of
    # adj / ea), producing aw fp16 [128, 2048] = [rt0 | rt1].
    aw_tiles = []
    for pair in range(2):
        base = 4 * half + 2 * pair
        band = 4 * g + 2 * half + pair
        hot = band == 0
        prio = pools["tc"].high_priority() if hot else contextlib.nullcontext()
        with prio:
            ea_t = pools["eap"].tile([_P, 2 * _N], pools["i32"])
            nc.sync.dma_start(
                out=ea_t[:].rearrange("p (q j) -> p q j", q=2),
                in_=ea_d[g, base * _P : (base + 2) * _P, :].rearrange(
                    "(q p) j -> p q j", p=_P
                ),
            )
            adj_t = pools["adjp"].tile([_P, 2 * _N], f32)
            nc.sync.dma_start(
                out=adj_t[:].rearrange("p (q j) -> p q j", q=2),
                in_=adj_d[g, base * _P : (base + 2) * _P, :].rearrange(
                    "(q p) j -> p q j", p=_P
                ),
            )

        if square_split == "act":
            sq_act = True
        elif square_split == "gpsimd":
            sq_act = False
        else:
            sq_act = band % 2 == 0

        if mode in ("cubic", "quad"):
            s_t = pools["sp"].tile([_P, 2 * _N], mmdt)
            if sq_act:
                # s = (a + h)^2 on ScalarE; add v2 later
                nc.scalar.activation(
                    s_t[:], ea_t[:], AF.Square,
                    bias=pools["hbias_sb"][:, 0:1], scale=1.0,
                )
                k_add = float(params["v2"])
            else:
                # s = (a - p) * a on GpSimd; add q later
                nc.gpsimd.scalar_tensor_tensor(
                    s_t[:], ea_t[:], float(params["p"]), ea_t[:],
                    OP.subtract, OP.mult,
                )
                k_add = float(params["q"])

        aw_t = pools["awp"].tile([_P, 2 * _N], mmdt)
        if mode == "cubic":
            qt_t = pools["qtp"].tile([_P, 2 * _N], mmdt)
            nc.vector.scalar_tensor_tensor(
                qt_t[:], ea_t[:], float(params["r"]), adj_t[:],
                OP.subtract, OP.mult,
            )
            nc.vector.scalar_tensor_tensor(
                aw_t[:], s_t[:], k_add, qt_t[:], OP.add, OP.mult
            )
        elif mode == "quad":
            nc.vector.scalar_tensor_tensor(
                aw_t[:], s_t[:], k_add, adj_t[:], OP.add, OP.mult
            )
        elif mode == "linear":
            nc.vector.scalar_tensor_tensor(
                aw_t[:], ea_t[:], float(params["r"]), adj_t[:],
                OP.subtract, OP.mult,
            )
        else:  # const
            nc.vector.tensor_copy(aw_t[:], adj_t[:])
        aw_tiles.append(aw_t)

    def aw_ap(k, jt):
        # k-th row-tile of this half (k in 0..3), j-tile jt: [128, 128]
        return aw_tiles[k // 2][:, (k % 2) * _N + jt * _P : (k % 2) * _N + (jt + 1) * _P]

    # Transpose aw blocks via regular matmuls (jt-pair batched into one PSUM
    # tile) and accumulate agg^T over j.
    p_aggT = pools["ps_agg"].tile([_C, _H], f32, tag="ps_agg")
    for jtp in range(4):
        p_tp = pools["ps_tp"].tile([_P, 2 * _H], f32, tag="ps_tp")
        for sub in range(2):
            jt = 2 * jtp + sub
            for k in range(4):
                nc.tensor.matmul(
                    p_tp[:, sub * _H + k * _P : sub * _H + (k + 1) * _P],
                    lhsT=aw_ap(k, jt),
                    rhs=ident_m[:, :_P],
                    start=True, stop=True,
                )
        awT = pools["awTp"].tile([_P, 2 * _H], mmdt)
        nc.scalar.copy(out=awT[:], in_=p_tp[:])
        for sub in range(2):
            jt = 2 * jtp + sub
            nc.tensor.matmul(
                p_aggT[:],
                lhsT=xs_mm[:, jt * _C : (jt + 1) * _C],
                rhs=awT[:, sub * _H : (sub + 1) * _H],
                start=(jt == 0),
                stop=False,
            )
    # b_rel enters the same accumulation group as a rank-1 (K=1) term.
    nc.tensor.matmul(
        p_aggT[:], lhsT=pools["brel_sb"][:], rhs=pools["ones_sb"][:],
        start=False, stop=True,
    )

    aggT = pools["aggTp"].tile([_C, _H], mmdt)
    nc.scalar.copy(out=aggT[:], in_=p_aggT[:])

    # out^T[c', i-half] = W_rel^T @ aggT (+ b_rel, already in) + W_root^T @ xT
    p_out = pools["ps_out"].tile([_C, _H], f32, tag="ps_out")
    nc.tensor.matmul(
        p_out[:], lhsT=pools["wrel_sb"][:], rhs=aggT[:], start=True, stop=False
    )
    nc.tensor.matmul(
        p_out[:], lhsT=pools["wroot_sb"][:],
        rhs=xT[:, half * _H : (half + 1) * _H],
        start=False, stop=True,
    )
    outT = pools["outTp"].tile([_C, _H], mmdt)
    nc.scalar.copy(out=outT[:], in_=p_out[:])

    # back to natural [i, c] layout (regular-mm transpose) and store
    p_on = pools["ps_out"].tile([_P, 4 * _C], f32, tag="ps_out")
    for k in range(4):
        nc.tensor.matmul(
            p_on[:, k * _C : (k + 1) * _C],
            lhsT=outT[:, k * _P : (k + 1) * _P],
            rhs=ident_m[:_C, :_C],
            start=True, stop=True,
        )
    out_sb = pools["outp"].tile([_P, 4 * _C], f32)
    nc.scalar.copy(out=out_sb[:], in_=p_on[:])
    nc.sync.dma_start(
        out=out_d[g, half * _H : (half + 1) * _H, :].rearrange(
            "(t p) c -> p t c", p=_P
        ),
        in_=out_sb[:].rearrange("p (t c) -> p t c", t=4),
    )


def _emit_graph(nc, tc, pools, g, dram, mode, params, square_split):
    f32 = pools["f32"]
    mmdt = pools["mmdt"]
    x_d = dram["x"]

    # x in aggregation layout: xs[p, t*C+c] = x[t*128+p, c]
    xs = pools["xsp"].tile([_P, _NT * _C], f32)
    nc.sync.dma_start(
        out=xs[:].rearrange("p (t c) -> p t c", t=_NT),
        in_=x_d[g, :, :].rearrange("(t p) c -> p t c", p=_P),
    )
    xs_mm = pools["xsp"].tile([_P, _NT * _C], mmdt, tag="xs_mm")
    nc.vector.tensor_copy(xs_mm[:], xs[:])
    xT = pools["xTp"].tile([_C, _N], mmdt)

    for half in range(2):
        _emit_half(
            nc, pools, g, half, dram, xs_mm, xT, mode, params, square_split
        )


def _build_module(mode, params, square_split, mm_dtype):
    import concourse.bass as bass  # noqa: F401
    from concourse import bacc, mybir
    from concourse.tile import TileContext

    f32 = mybir.dt.float32
    i32 = mybir.dt.int32
    mmdt = getattr(mybir.dt, mm_dtype)

    nc = bacc.Bacc(
        "TRN2", target_bir_lowering=False, debug=False, num_devices=_NCORES
    )

    dram = {
        "x": nc.dram_tensor("x", [_G, _N, _C], f32, kind="ExternalInput"),
        "adj": nc.dram_tensor("adj", [_G, _N, _N], f32, kind="ExternalInput"),
        "ea": nc.dram_tensor("ea", [_G, _N, _N], i32, kind="ExternalInput"),
        "wrel": nc.dram_tensor("wrel", [_C, _C], f32, kind="ExternalInput"),
        "wroot": nc.dram_tensor("wroot", [_C, _C], f32, kind="ExternalInput"),
        "brel": nc.dram_tensor("brel", [1, _C], f32, kind="ExternalInput"),
        "ident": nc.dram_tensor("ident", [_P, _P], f32, kind="ExternalInput"),
        "out": nc.dram_tensor("out", [_G, _N, _C], f32, kind="ExternalOutput"),
    }

    pool_specs = [
        ("consts", 1, None),
        ("adjp", 5, None),
        ("eap", 5, None),
        ("sp", 3, None),
        ("qtp", 3, None),
        ("awp", 3, None),
        ("awTp", 3, None),
        ("xsp", 2, None),
        ("xTp", 1, None),
        ("aggTp", 2, None),
        ("outTp", 2, None),
        ("outp", 2, None),
        ("ps_tp", 2, "PSUM"),
        ("ps_agg", 2, "PSUM"),
        ("ps_xt", 1, "PSUM"),
        ("ps_out", 1, "PSUM"),
    ]

    with TileContext(nc) as tc, ExitStack() as ctx:
        pools = {"f32": f32, "i32": i32, "mmdt": mmdt, "tc": tc}
        for name, bufs, space in pool_specs:
            kw = {"space": space} if space else {}
            pools[name] = ctx.enter_context(tc.tile_pool(name=name, bufs=bufs, **kw))

        ident = pools["consts"].tile([_P, _P], f32, tag="ident")
        nc.sync.dma_start(out=ident[:], in_=dram["ident"][:, :])
        pools["ident"] = ident
        ident_m = pools["consts"].tile([_P, _P], mmdt, tag="ident_m")
        nc.vector.tensor_copy(ident_m[:], ident[:])
        pools["ident_m"] = ident_m
        for wname in ("wrel", "wroot"):
            t = pools["consts"].tile([_C, _C], f32, tag=wname)
            nc.sync.dma_start(out=t[:], in_=dram[wname][:, :])
            tf = pools["consts"].tile([_C, _C], mmdt, tag=wname + "_f")
            nc.vector.tensor_copy(tf[:], t[:])
            pools[wname + "_sb"] = tf
        tb = pools["consts"].tile([1, _C], f32, tag="brel32")
        nc.sync.dma_start(out=tb[:], in_=dram["brel"][:, :])
        brel = pools["consts"].tile([1, _C], mmdt, tag="brel")
        nc.vector.tensor_copy(brel[:], tb[:])
        pools["brel_sb"] = brel
        ones = pools["consts"].tile([1, _H], mmdt, tag="ones")
        nc.vector.memset(ones[:], 1.0)
        pools["ones_sb"] = ones

        if mode in ("cubic", "quad"):
            hb = pools["consts"].tile([_P, 1], f32, tag="hb")
            nc.vector.memset(hb[:], float(params["h"]))
            pools["hbias_sb"] = hb

        for g in range(_G):
            _emit_graph(nc, tc, pools, g, dram, mode, params, square_split)

    nc.finalize()
    return nc


def _get_module(w_edge, square_split, mm_dtype):
    mode, params, lead = _chain_params(w_edge)
    key = (
        mode,
        tuple(sorted((k, round(v, 15)) for k, v in params.items())),
        square_split,
        mm_dtype,
    )
    if key not in _BUILD_CACHE:
        _BUILD_CACHE[key] = _build_module(mode, params, square_split, mm_dtype)
    return _BUILD_CACHE[key], lead


def _prep_inputs(x, adj, edge_attr, W_rel, b_rel, W_root, w_edge):
    x = np.ascontiguousarray(np.asarray(x, dtype=np.float32))
    adj = np.ascontiguousarray(np.asarray(adj, dtype=np.float32))
    ea = np.ascontiguousarray(np.asarray(edge_attr, dtype=np.int32).reshape(_B, _N, _N))
    W_rel = np.asarray(W_rel, dtype=np.float64)
    W_root = np.ascontiguousarray(np.asarray(W_root, dtype=np.float32))
    b_rel = np.asarray(b_rel, dtype=np.float32).reshape(1, _C)
    w_edge = np.asarray(w_edge)
    return x, adj, ea, W_rel, b_rel, W_root, w_edge


def kernel(x, adj, edge_attr, W_rel, b_rel, W_root, w_edge):
    global LAST_RESULTS
    from concourse.bass_utils import run_bass_kernel_spmd

    x, adj, ea, W_rel, b_rel, W_root, w_edge = _prep_inputs(
        x, adj, edge_attr, W_rel, b_rel, W_root, w_edge
    )
    nc, lead = _get_module(w_edge, SQUARE_SPLIT, MM_DTYPE)
    wrel_eff = np.ascontiguousarray((lead * W_rel).astype(np.float32))
    ident = np.eye(_P, dtype=np.float32)

    in_maps = []
    for c in range(_NCORES):
        sl = slice(c * _G, (c + 1) * _G)
        in_maps.append(
            {
                "x": x[sl],
                "adj": adj[sl],
                "ea": ea[sl],
                "wrel": wrel_eff,
                "wroot": W_root,
                "brel": b_rel,
                "ident": ident,
            }
        )

    res = run_bass_kernel_spmd(nc, in_maps, list(range(_NCORES)), trace=TRACE)
    LAST_RESULTS = res
    out = np.concatenate([res.results[c]["out"] for c in range(_NCORES)], axis=0)
    return out
